# revision 22
# baseline (speedup 1.0000x reference)
"""Trainium2 Bass kernel for nn_BackBone_77532749627801.

Transformer encoder block: per-head QKV projections -> MHA (softmax over
keys) -> AddNorm -> FFN (erf GELU) -> AddNorm.  B=4, S=2048, D=1024, H=16,
DH=64, F=4096.

Sharding: 8 cores = 4 batches x 2 sequence-halves; each core runs the block
for 1024 query tokens of one batch (K/V computed for the full 2048-token
sequence, so no collectives).  Host-side rotation puts each core's own
tokens in xT columns 0:1024.

Schedule: the softmax exp stream (33.6M elements) pins the activation
engine at ~250us, while the matmul stream pins the PE at ~530us, so the
kernel is organized to overlap them.  Window 1 computes QKV projections and
attention for query-half 0, with projections prefetched two head-pairs
ahead so the exp stream never starves (K/Q PSUM evictions run on the DVE to
keep the act engine on exp).  Window 2 runs attention for query-half 1 with
FFN1 chunks for half 0 interleaved between the score matmuls; the half-0
gelus are deferred out of the exp stream (activation-table swaps cost
1.3us) and staged through aT's half-1 slots.  Window 3 is PE-bound: FFN1
half 1 with the deferred gelus interleaved, then FFN2 per half with the b2
bias folded in via a rank-1 ones matmul and the LN2 + output DMA fused per
half.

v4 refinements (each HW-validated correct at 8.52e-3, timing within run
noise of the 745-780us band): V projection restructured so quad pairs
share each stationary x-tile through a single 2-bank PSUM tile (keeps
the scheduler from splitting the chains) and a post-schedule pass strips
the then-redundant LDWEIGHTS (397 total, incl. FFN2's nh-pairs); the
LayerNorm rsqrt moved off the ACT engine (bit-trick + 2 Newton steps on
the DVE) so exp/gelu activation tables never swap against Sqrt; xT DMA
split per d-chunk so the first projections wait only on data they read.
Measured flat => the PE stream is already near its practical occupancy
for this schedule; LDWEIGHTS were being hidden by the background weight
buffer, and act-table swaps were off the critical path.

Precision (gate 2e-2; measured 8.5e-3 end-to-end on HW): QKV projections
and attn@V in fp8e4m3 DoubleRow (two 128-partition contraction tiles per
pass, 2x bf16 throughput - measured; the cost model's additional 2x for
DoubleRow does not materialize on hardware, the 256B/cycle input port is
the ceiling); q/k/exp(scores) stored fp8e4m3; scores and FFN matmuls in
bf16; PSUM always fp32; layernorm stats in fp32 over bf16 activations.
The softmax denominator falls out of attn@V via a ones-column on V (padded
to 68-wide blocks - dual-fp8 Ldweights requires 4-aligned stationary
columns); biases fold into matmuls or evictions; softmax uses a constant
shift (scores are O(+-6)).
"""

import contextlib
import os
import sys

import numpy as np

if "/opt/trn_rl_repo" not in sys.path and os.path.isdir("/opt/trn_rl_repo"):
    sys.path.insert(0, "/opt/trn_rl_repo")

B, S, D, H, DH, F = 4, 2048, 1024, 16, 64, 4096
N_CORES = 8
TOK = 1024  # query tokens per core
EPS = 1e-5
EXP_SHIFT = -3.0  # constant shift inside exp; cancels in softmax

_BUILD_CACHE = {}


def _strip_redundant_ldweights(nc, mybir):
    """Remove InstLdweights whose stationary operand is identical to the
    immediately preceding weight load (PE retains the stationary operand
    across matmuls).  Runs post-scheduling, pre-compile; only drops loads
    that carry no sync and whose key (AP+mode) matches the previous load
    with nothing but matmuls in between."""
    removed = 0
    for bb in nc.main_func.blocks:
        new = []
        last = None
        for inst in bb.instructions:
            if isinstance(inst, mybir.InstLdweights):
                pap = inst.ins[0]
                key = (pap.memref, pap.offset, str(pap.ap), str(pap.dtype),
                       str(inst.perf_mode), bool(inst.is_transpose),
                       str(inst.tile_position))
                si = inst.sync_info
                clean = si is None or (not si.on_wait and not si.on_update)
                if last == key and clean:
                    removed += 1
                    continue
                last = key
            elif isinstance(
                    inst, (mybir.InstMatmult, mybir.InstEventSemaphore,
                           mybir.InstNoOp)):
                # matmuls consume (not clobber) the loaded weights;
                # event-semaphores/nops only wait — all transparent to the
                # PE array's weight state
                pass
            elif getattr(inst, "engine", None) == mybir.EngineType.PE:
                last = None
            new.append(inst)
        bb.instructions = new
    return removed


def _build(n_iters=1, use_dp=True):
    import concourse.bacc as bacc
    import concourse.mybir as mybir
    import concourse.tile as tile
    from concourse.masks import make_identity
    from contextlib import ExitStack

    f32 = mybir.dt.float32
    bf16 = mybir.dt.bfloat16
    e4 = mybir.dt.float8e4
    AF = mybir.ActivationFunctionType
    DR = mybir.MatmulPerfMode.DoubleRow
    DP = mybir.MatmulPerfMode.DoublePixel if use_dp else None

    nc = bacc.Bacc("TRN2", target_bir_lowering=False, debug=False,
                   num_devices=N_CORES)

    xT = nc.dram_tensor("xT", [D, S], e4, kind="ExternalInput").ap()
    xh = nc.dram_tensor("xh", [TOK, D], bf16, kind="ExternalInput").ap()
    wq = nc.dram_tensor("wq", [D, D], e4, kind="ExternalInput").ap()
    wk = nc.dram_tensor("wk", [D, D], e4, kind="ExternalInput").ap()
    wv = nc.dram_tensor("wv", [D, 4, 272], e4, kind="ExternalInput").ap()
    bqk = nc.dram_tensor("bqk", [2, D], f32, kind="ExternalInput").ap()
    bv4 = nc.dram_tensor("bv4", [1, 4, 272], f32, kind="ExternalInput").ap()
    w1 = nc.dram_tensor("w1", [D, F], bf16, kind="ExternalInput").ap()
    b1d = nc.dram_tensor("b1", [F], f32, kind="ExternalInput").ap()
    w2 = nc.dram_tensor("w2", [F, D], bf16, kind="ExternalInput").ap()
    b2rh = nc.dram_tensor("b2rh", [1, D], bf16, kind="ExternalInput").ap()
    ln1g = nc.dram_tensor("ln1g", [D], bf16, kind="ExternalInput").ap()
    ln2g = nc.dram_tensor("ln2g", [D], bf16, kind="ExternalInput").ap()
    ln2b = nc.dram_tensor("ln2b", [D], bf16, kind="ExternalInput").ap()
    out = nc.dram_tensor("out", [TOK, D], f32, kind="ExternalOutput").ap()

    with tile.TileContext(nc) as tc, ExitStack() as top:
        const = top.enter_context(tc.tile_pool(name="const", bufs=1))
        ident_h = const.tile([128, 128], bf16)
        make_identity(nc, ident_h)
        eshift = const.tile([128, 1], f32)
        nc.vector.memset(eshift, EXP_SHIFT)
        eps_t = const.tile([128, 1], f32)
        nc.vector.memset(eps_t, EPS)
        bq_sb = const.tile([128, 8], f32)
        nc.sync.dma_start(out=bq_sb, in_=bqk[0].rearrange("(pr p) -> p pr", p=128))
        bk_sb = const.tile([128, 8], f32)
        nc.sync.dma_start(out=bk_sb, in_=bqk[1].rearrange("(pr p) -> p pr", p=128))
        b1_sb = const.tile([128, 32], f32)
        nc.sync.dma_start(out=b1_sb, in_=b1d.rearrange("(fc p) -> p fc", p=128))
        ones_r = const.tile([1, 128], bf16)
        nc.vector.memset(ones_r, 1.0)

        resid = top.enter_context(tc.tile_pool(name="resid", bufs=1))

        loop = tc.For_i(0, n_iters) if n_iters > 1 else contextlib.nullcontext()
        with loop:
            mha = resid.tile([128, 8, D], bf16, tag="mha")

            with ExitStack() as pha:
                att = pha.enter_context(tc.tile_pool(name="att", bufs=1))
                zpool = pha.enter_context(tc.tile_pool(name="zp", bufs=1))
                stream = pha.enter_context(tc.tile_pool(name="stream", bufs=2))
                stat = pha.enter_context(tc.tile_pool(name="stat", bufs=4))
                opool = pha.enter_context(tc.tile_pool(name="osb", bufs=1))

                kT_all = att.tile([128, 8, S], e4)
                qT_all = att.tile([128, 8, TOK], e4)
                vplus_all = att.tile([128, 4, 16, 272], e4)
                h1T = att.tile([128, 8, TOK], bf16)

                with tc.tile_pool(name="psA", bufs=1, space="PSUM") as psA:

                    def scores_part(head, sch, hook=None):
                        pair, h01 = head // 2, head % 2
                        pslice = slice(h01 * 64, h01 * 64 + 64)
                        expT = att.tile([128, 16, 512], e4, tag="expT",
                                        bufs=2)
                        for tp in range(8):
                            sT_ps = psA.tile([128, 1024], f32, tag="sT2",
                                             bufs=2)
                            for sub in range(2):
                                tt = 2 * tp + sub
                                nc.tensor.matmul(
                                    out=sT_ps[:, sub * 512:(sub + 1) * 512],
                                    lhsT=kT_all[pslice, pair,
                                                tt * 128:(tt + 1) * 128],
                                    rhs=qT_all[pslice, pair,
                                               sch * 512:(sch + 1) * 512],
                                    start=True, stop=True,
                                    perf_mode=DP)
                            nc.scalar.activation(
                                out=expT[:, 2 * tp:2 * tp + 2, :],
                                in_=sT_ps, func=AF.Exp,
                                bias=eshift[:, :], scale=0.125)
                            if hook is not None:
                                hook(tp)
                        return expT

                    def attnv_part(head, sch, expT):
                        pair, h01 = head // 2, head % 2
                        quad, j = pair // 2, 2 * (pair % 2) + h01
                        zT_ps = psA.tile([68, 512], f32, tag="zT", bufs=1)
                        for t2 in range(8):
                            nc.tensor.matmul(
                                out=zT_ps,
                                lhsT=vplus_all[:, quad, 2 * t2:2 * t2 + 2,
                                               68 * j:68 * j + 68],
                                rhs=expT[:, 2 * t2:2 * t2 + 2, :],
                                start=(t2 == 0), stop=(t2 == 7),
                                perf_mode=DR)
                        zT_sb = zpool.tile([68, 512], bf16, tag="zT_sb")
                        nc.vector.tensor_copy(out=zT_sb, in_=zT_ps)
                        for sb4 in range(4):
                            ztr = psA.tile([128, 68], bf16, tag="small",
                                           bufs=1)
                            nc.tensor.transpose(
                                out=ztr,
                                in_=zT_sb[:, sb4 * 128:(sb4 + 1) * 128],
                                identity=ident_h[0:68, 0:68])
                            rec = zpool.tile([128, 1], f32, tag="rec")
                            nc.vector.reciprocal(out=rec, in_=ztr[:, 64:65])
                            stg = sch * 4 + sb4
                            nc.vector.tensor_scalar_mul(
                                out=mha[:, stg, head * 64:head * 64 + 64],
                                in0=ztr[:, 0:64], scalar1=rec)

                    def attn_block(head, sch, hook=None):
                        attnv_part(head, sch, scores_part(head, sch, hook))

                    ph1 = ExitStack()
                    wp = ph1.enter_context(tc.tile_pool(name="wpair", bufs=2))

                    # xT lives in the iteration-long att pool (NOT the
                    # early-closed ph1 pools): if its SBUF aliased aT, the
                    # next For_i iteration's xT DMA would serialize behind
                    # the entire FFN2 window; de-aliasing lets the reload
                    # overlap this iteration's LN2 tail
                    xT_sb = att.tile([128, 8, S], e4, tag="xTsb")
                    # split across both DMA queues AND per d-chunk so the
                    # first projection matmuls wait only on the chunks
                    # they actually read, not the full 2MB
                    for dt in range(4):
                        nc.sync.dma_start(
                            out=xT_sb[:, dt, :],
                            in_=xT.rearrange("(dt p) s -> p dt s",
                                             p=128)[:, dt, :])
                        nc.gpsimd.dma_start(
                            out=xT_sb[:, 4 + dt, :],
                            in_=xT.rearrange("(dt p) s -> p dt s",
                                             p=128)[:, 4 + dt, :])

                    # V biases + residual input + LN constants ride the
                    # gpsimd DMA queue so the sync queue serves xT / weights
                    bv_all4 = wp.tile([128, 4, 272], f32, tag="bv4",
                                      bufs=1)
                    nc.gpsimd.dma_start(
                        out=bv_all4, in_=bv4[0, :, :].partition_broadcast(128))
                    xh_sb = resid.tile([128, 8, D], bf16, tag="xh")
                    g1_bc = resid.tile([128, D], bf16, tag="g1")
                    nc.gpsimd.dma_start(out=g1_bc,
                                        in_=ln1g.partition_broadcast(128))
                    g2_bc = resid.tile([128, D], bf16, tag="g2")
                    nc.gpsimd.dma_start(out=g2_bc,
                                        in_=ln2g.partition_broadcast(128))
                    b2_bc = resid.tile([128, D], bf16, tag="lb2")
                    nc.gpsimd.dma_start(out=b2_bc,
                                        in_=ln2b.partition_broadcast(128))
                    b2row = resid.tile([1, D], bf16, tag="b2row")
                    nc.gpsimd.dma_start(out=b2row, in_=b2rh[:, :])

                    # ---- window 1: projections + attention(half 0) ------
                    wvs = {}

                    def v_dma(quad):
                        wv_sb = wp.tile([128, 8, 272], e4, tag="wv", bufs=4)
                        nc.sync.dma_start(
                            out=wv_sb,
                            in_=wv[:, quad, :].rearrange(
                                "(dt p) c -> p dt c", p=128))
                        wvs[quad] = wv_sb

                    def v_piece_pair(qp, tt):
                        # quads 2qp/2qp+1 share each stationary x-tile.
                        # Both accumulation chains live in ONE 2-bank PSUM
                        # tile so they become ready together and the
                        # scheduler keeps their matmuls interleaved; the
                        # then-redundant weight reloads (213ns each, vs
                        # 57ns of streaming) get stripped post-schedule.
                        quads = (2 * qp, 2 * qp + 1)
                        vp = psA.tile([128, 2, 512], f32, tag="sT2",
                                      bufs=2, name=f"vpp_{qp}_{tt}")
                        for dp in range(4):
                            for i in range(2):
                                nc.tensor.matmul(
                                    out=vp[:, i, 0:272],
                                    lhsT=xT_sb[:, 2 * dp:2 * dp + 2,
                                               tt * 128:(tt + 1) * 128],
                                    rhs=wvs[quads[i]][:, 2 * dp:2 * dp + 2, :],
                                    start=(dp == 0), stop=(dp == 3),
                                    perf_mode=DR)
                        for i in range(2):
                            nc.vector.tensor_add(
                                out=vplus_all[:, quads[i], tt, :],
                                in0=vp[:, i, 0:272],
                                in1=bv_all4[:, quads[i], :])

                    def v_piece_all(tt):
                        v_piece_pair(0, tt)
                        v_piece_pair(1, tt)

                    def proj_pair(pair):
                        proj_pair_k(pair)
                        proj_pair_q(pair)

                    wqs = {}

                    def proj_pair_k(pair):
                        wk_sb = wp.tile([128, 8, 128], e4, tag="wk")
                        nc.sync.dma_start(
                            out=wk_sb,
                            in_=wk[:, pair * 128:(pair + 1) * 128]
                            .rearrange("(dt p) m -> p dt m", p=128))
                        wq_sb = wp.tile([128, 8, 128], e4, tag="wq")
                        nc.sync.dma_start(
                            out=wq_sb,
                            in_=wq[:, pair * 128:(pair + 1) * 128]
                            .rearrange("(dt p) m -> p dt m", p=128))
                        wqs[pair] = wq_sb
                        for ch in range(2):
                            kq_ps = psA.tile([128, 1024], f32,
                                             tag="sT2", bufs=2)
                            for nh in range(2):
                                for dp in range(4):
                                    nc.tensor.matmul(
                                        out=kq_ps[:, nh * 512:
                                                  (nh + 1) * 512],
                                        lhsT=wk_sb[:, 2 * dp:2 * dp + 2, :],
                                        rhs=xT_sb[:, 2 * dp:2 * dp + 2,
                                                  (2 * ch + nh) * 512:
                                                  (2 * ch + nh + 1) * 512],
                                        start=(dp == 0), stop=(dp == 3),
                                        perf_mode=DR)
                            nc.vector.tensor_scalar_add(
                                out=kT_all[:, pair,
                                           ch * 1024:(ch + 1) * 1024],
                                in0=kq_ps,
                                scalar1=bk_sb[:, pair:pair + 1])

                    def proj_pair_q(pair):
                        wq_sb = wqs.pop(pair)
                        kq_ps = psA.tile([128, 1024], f32, tag="sT2",
                                         bufs=2)
                        for nh in range(2):
                            for dp in range(4):
                                nc.tensor.matmul(
                                    out=kq_ps[:, nh * 512:(nh + 1) * 512],
                                    lhsT=wq_sb[:, 2 * dp:2 * dp + 2, :],
                                    rhs=xT_sb[:, 2 * dp:2 * dp + 2,
                                              nh * 512:(nh + 1) * 512],
                                    start=(dp == 0), stop=(dp == 3),
                                    perf_mode=DR)
                        nc.vector.tensor_scalar_add(
                            out=qT_all[:, pair, :], in0=kq_ps,
                            scalar1=bq_sb[:, pair:pair + 1])

                    # two-pair-ahead prefetch keeps the exp stream
                    # running; projection pieces are scattered between the
                    # score groups so the PE never sits >4us ahead of the
                    # act engine's exp drain.  All 16 V tt-pieces complete
                    # before the first attn@V (which consumes the full key
                    # range): tt 0-7 in the prologue, 8-15 inside pair 0.
                    proj_pair(0)
                    for quad in range(4):
                        v_dma(quad)
                    for tt in range(8):
                        v_piece_all(tt)
                    proj_pair(1)
                    for pair in range(8):
                        if pair == 4:
                            for st in range(8):
                                nc.gpsimd.dma_start(
                                    out=xh_sb[:, st, :],
                                    in_=xh[st * 128:(st + 1) * 128, :])
                        nxt = pair + 2
                        eT0 = scores_part(2 * pair, 0)
                        if pair == 0:
                            for tt in range(8, 12):
                                v_piece_all(tt)
                        if nxt < 8:
                            proj_pair_k(nxt)
                        eT1 = scores_part(2 * pair + 1, 0)
                        if pair == 0:
                            for tt in range(12, 16):
                                v_piece_all(tt)
                        if nxt < 8:
                            proj_pair_q(nxt)
                        attnv_part(2 * pair, 0, eT0)
                        attnv_part(2 * pair + 1, 0, eT1)

                    ph1.close()  # free xT + weight staging for aT

                    # ---- LN1 + h1T transposes + g1 fold, per half -------
                    def layer_norm_tile(st, g_bc, b_bc, add_in=None,
                                        add_ps=None, out_dma=False):
                        h = mha[:, st, :]
                        if add_in is not None:
                            nc.vector.tensor_add(out=h, in0=h, in1=add_in)
                        if add_ps is not None:
                            nc.vector.tensor_add(out=h, in0=h, in1=add_ps)
                        stats = stat.tile([128, 2, 6], f32, tag="stats")
                        for sg in range(2):
                            nc.vector.bn_stats(
                                out=stats[:, sg, :],
                                in_=h[:, sg * 512:(sg + 1) * 512])
                        mv = stat.tile([128, 2], f32, tag="mv")
                        nc.vector.bn_aggr(out=mv, in_=stats)
                        # rsqrt(var+eps) on the DVE (bit-trick seed + 2
                        # Newton steps, ~5e-6 rel err): keeps the Sqrt off
                        # the ACT engine, whose exp/gelu tables would
                        # otherwise thrash (no table set holds exp+sqrt;
                        # each swap costs ~1.3us and stalls the LN chain)
                        i32 = mybir.dt.int32
                        AL = mybir.AluOpType
                        vv = mv[:, 1:2]
                        ti = stat.tile([128, 1], i32, tag="ti")
                        qq = stat.tile([128, 1], f32, tag="qq")
                        y1 = stat.tile([128, 1], f32, tag="y1")
                        nc.vector.tensor_scalar(out=vv, in0=vv, scalar1=EPS,
                                                scalar2=None, op0=AL.add)
                        nc.vector.tensor_scalar(
                            out=ti, in0=vv.bitcast(i32), scalar1=1,
                            scalar2=None, op0=AL.logical_shift_right)
                        nc.vector.tensor_scalar(
                            out=ti, in0=ti, scalar1=-1, scalar2=0x5f3759df,
                            op0=AL.mult, op1=AL.add)
                        for nit in range(2):
                            yin = ti.bitcast(f32) if nit == 0 else y1
                            yout = y1 if nit == 0 else mv[:, 1:2]
                            nc.vector.tensor_tensor(out=qq, in0=vv, in1=yin,
                                                    op=AL.mult)
                            nc.vector.tensor_tensor(out=qq, in0=qq, in1=yin,
                                                    op=AL.mult)
                            nc.vector.tensor_scalar(
                                out=qq, in0=qq, scalar1=-0.5, scalar2=1.5,
                                op0=AL.mult, op1=AL.add)
                            nc.vector.tensor_tensor(out=yout, in0=yin,
                                                    in1=qq, op=AL.mult)
                        nc.vector.tensor_scalar(
                            out=h, in0=h, scalar1=mv[:, 0:1],
                            scalar2=mv[:, 1:2],
                            op0=mybir.AluOpType.subtract,
                            op1=mybir.AluOpType.mult)
                        if g_bc is not None:
                            nc.vector.tensor_mul(out=h, in0=h, in1=g_bc[:, :])
                            o_sb = opool.tile([128, D], f32, tag="osb")
                            nc.vector.tensor_add(out=o_sb, in0=h,
                                                 in1=b_bc[:, :])
                            if out_dma:
                                nc.sync.dma_start(
                                    out=out.rearrange(
                                        "(st p) d -> p st d", p=128)[:, st, :],
                                    in_=o_sb)

                    def b1_half(half):
                        for st in range(4 * half, 4 * half + 4):
                            layer_norm_tile(st, None, None,
                                            add_in=xh_sb[:, st, :])
                        for st in range(4 * half, 4 * half + 4):
                            for dt in range(8):
                                # alternate between the two idle PSUM banks
                                # (vp/ztr and zT are unused here) so the
                                # transpose->copy chains double-buffer
                                tr_ps = psA.tile([128, 128], bf16,
                                                 tag=("small" if dt % 2 == 0
                                                      else "zT"), bufs=1)
                                nc.tensor.transpose(
                                    out=tr_ps,
                                    in_=mha[:, st, dt * 128:(dt + 1) * 128],
                                    identity=ident_h[:, :])
                                nc.vector.tensor_copy(
                                    out=h1T[:, dt, st * 128:(st + 1) * 128],
                                    in_=tr_ps)
                        for st in range(4 * half, 4 * half + 4):
                            nc.vector.tensor_mul(out=mha[:, st, :],
                                                 in0=mha[:, st, :],
                                                 in1=g1_bc[:, :])

                    b1_half(0)

                    # ---- window 2: attention(half 1) + FFN1 half 0 ------
                    aTp = pha.enter_context(tc.tile_pool(name="aTp", bufs=1))
                    aT = aTp.tile([128, 32, TOK], bf16, tag="aT")
                    fstate = {}

                    def ffn1_chunk(fc, half, dts):
                        if dts[0] == 0:
                            w1t = stream.tile([128, 8, 128], bf16, tag="w1t",
                                              name=f"w1t_{half}_{fc}")
                            nc.sync.dma_start(
                                out=w1t,
                                in_=w1[:, fc * 128:(fc + 1) * 128].rearrange(
                                    "(dt p) f -> p dt f", p=128))
                            fstate["w1t"] = w1t
                            fstate["aps"] = psA.tile(
                                [128, 512], f32, tag="aps", bufs=2,
                                name=f"aps_{half}_{fc}")
                        a_ps = fstate["aps"]
                        for dt in dts:
                            nc.tensor.matmul(
                                out=a_ps,
                                lhsT=fstate["w1t"][:, dt, :],
                                rhs=h1T[:, dt,
                                        half * 512:(half + 1) * 512],
                                start=(dt == 0), stop=(dt == 7))
                        if dts[-1] == 7:
                            if half == 0:
                                # defer gelu out of the exp stream (the act
                                # table swap costs 1.3us); stage raw preact
                                # in the unused half-1 slot of aT
                                nc.vector.tensor_copy(
                                    out=aT[:, fc, 512:1024], in_=a_ps)
                            else:
                                nc.scalar.activation(
                                    out=aT[:, fc, 512:1024],
                                    in_=a_ps, func=AF.Gelu,
                                    bias=b1_sb[:, fc:fc + 1])

                    def w2_hook(head):
                        def hook(tp):
                            fc = 2 * head + tp // 4
                            dts = [2 * (tp % 4), 2 * (tp % 4) + 1]
                            ffn1_chunk(fc, 0, dts)
                        return hook

                    for head in range(16):
                        attn_block(head, 1, hook=w2_hook(head))

                    b1_half(1)

                    # ---- window 3a: FFN1 half 1, with the deferred half-0
                    # gelus interleaved per fc (same act table, no swaps)
                    for fc in range(32):
                        nc.scalar.activation(
                            out=aT[:, fc, 0:512], in_=aT[:, fc, 512:1024],
                            func=AF.Gelu, bias=b1_sb[:, fc:fc + 1])
                        for dts in ([0, 1], [2, 3], [4, 5], [6, 7]):
                            ffn1_chunk(fc, 1, dts)

                # ---- window 3b: FFN2 per half + fused LN2 + DMA ---------
                with tc.tile_pool(name="ps2", bufs=1, space="PSUM") as ps2:
                    for sh in range(2):
                        ffps = [ps2.tile([128, D], f32, tag="ff", bufs=4,
                                         name=f"ff_{sh}_{i}")
                                for i in range(4)]
                        for fc in range(32):
                            w2t = stream.tile([128, D], bf16, tag="w2t")
                            nc.sync.dma_start(
                                out=w2t, in_=w2[fc * 128:(fc + 1) * 128, :])
                            for st2 in range(4):
                                base = sh * 512 + st2 * 128
                                for nh in range(2):
                                    nc.tensor.matmul(
                                        out=ffps[st2][:, nh * 512:
                                                      (nh + 1) * 512],
                                        lhsT=aT[:, fc, base:base + 128],
                                        rhs=w2t[:, nh * 512:(nh + 1) * 512],
                                        start=(fc == 0), stop=False)
                        for st2 in range(4):
                            for nh in range(2):
                                nc.tensor.matmul(
                                    out=ffps[st2][:, nh * 512:(nh + 1) * 512],
                                    lhsT=ones_r[:, :],
                                    rhs=b2row[:, nh * 512:(nh + 1) * 512],
                                    start=False, stop=True)
                        for st2 in range(4):
                            layer_norm_tile(sh * 4 + st2, g2_bc, b2_bc,
                                            add_ps=ffps[st2][:, :],
                                            out_dma=True)

    _strip_redundant_ldweights(nc, mybir)
    nc.compile()
    return nc


def _pack_inputs(x, Wq, bq, Wk, bk, Wv, bv, ln1_g, ln1_b, W1, b1, W2, b2,
                 ln2_g, ln2_b):
    """Build the 8 per-core input maps (host-side, numpy)."""
    from concourse import mybir

    f = np.float32
    E4 = mybir.dt.np(mybir.dt.float8e4)
    BF = mybir.dt.np(mybir.dt.bfloat16)
    wq_all = np.ascontiguousarray(
        np.transpose(np.asarray(Wq, f), (1, 0, 2)).reshape(D, D)).astype(E4)
    wk_all = np.ascontiguousarray(
        np.transpose(np.asarray(Wk, f), (1, 0, 2)).reshape(D, D)).astype(E4)
    Wv_ = np.asarray(Wv, f)
    bv_ = np.asarray(bv, f)
    wv_all = np.zeros((D, 4, 272), f)
    bv_all = np.zeros((1, 4, 272), f)
    for quad in range(4):
        for j in range(4):
            h = quad * 4 + j
            wv_all[:, quad, 68 * j:68 * j + 64] = Wv_[h]
            bv_all[0, quad, 68 * j:68 * j + 64] = bv_[h]
            bv_all[0, quad, 68 * j + 64] = 1.0
    bqk = np.stack([np.asarray(bq, f).reshape(D), np.asarray(bk, f).reshape(D)])
    x = np.asarray(x, f)
    W1_ = np.asarray(W1, np.float64)
    g1_ = np.asarray(ln1_g, np.float64)
    bb1_ = np.asarray(ln1_b, np.float64)
    w1_folded = (g1_[:, None] * W1_).astype(BF)
    b1_folded = (np.asarray(b1, np.float64) + bb1_ @ W1_).astype(f)
    b2_folded = (np.asarray(b2, np.float64) + bb1_).astype(f)
    common = dict(
        wq=wq_all, wk=wk_all, wv=wv_all.astype(E4), bqk=bqk, bv4=bv_all,
        w1=w1_folded, b1=b1_folded, w2=np.asarray(W2, f).astype(BF),
        b2rh=b2_folded.reshape(1, D).astype(BF),
        ln1g=np.asarray(ln1_g, f).astype(BF),
        ln2g=np.asarray(ln2_g, f).astype(BF),
        ln2b=np.asarray(ln2_b, f).astype(BF))
    in_maps = []
    for c in range(N_CORES):
        b_, half = c // 2, c % 2
        m = dict(common)
        own = x[b_, half * TOK:(half + 1) * TOK]
        other = x[b_, (1 - half) * TOK:(2 - half) * TOK]
        m["xT"] = np.ascontiguousarray(
            np.concatenate([own, other], axis=0).T).astype(E4)
        m["xh"] = np.ascontiguousarray(own).astype(BF)
        in_maps.append(m)
    return in_maps


def kernel(**inputs):
    from concourse.bass_utils import run_bass_kernel_spmd

    if "nc" not in _BUILD_CACHE:
        _BUILD_CACHE["nc"] = _build()
    nc = _BUILD_CACHE["nc"]
    in_maps = _pack_inputs(**inputs)
    res = run_bass_kernel_spmd(nc, in_maps, core_ids=list(range(N_CORES)))
    out = np.zeros((B, S, D), np.float32)
    for c in range(N_CORES):
        b_, half = c // 2, c % 2
        out[b_, half * TOK:(half + 1) * TOK] = res.results[c]["out"]
    return out



# revision 25
# speedup vs baseline: 1.0675x; 1.0675x over previous
"""Trainium2 Bass kernel for nn_BackBone_77532749627801.

Transformer encoder block: per-head QKV projections -> MHA (softmax over
keys) -> AddNorm -> FFN (erf GELU) -> AddNorm.  B=4, S=2048, D=1024, H=16,
DH=64, F=4096.

Sharding: 8 cores = 4 batches x 2 sequence-halves; each core runs the block
for 1024 query tokens of one batch (K/V computed for the full 2048-token
sequence, so no collectives).  Host-side rotation puts each core's own
tokens in xT columns 0:1024.

Schedule: the softmax exp stream (33.6M elements) pins the activation
engine at ~250us, while the matmul stream pins the PE at ~530us, so the
kernel is organized to overlap them.  Window 1 computes QKV projections and
attention for query-half 0, with projections prefetched two head-pairs
ahead so the exp stream never starves (K/Q PSUM evictions run on the DVE to
keep the act engine on exp).  Window 2 runs attention for query-half 1 with
FFN1 chunks for half 0 interleaved between the score matmuls; the half-0
gelus are deferred out of the exp stream (activation-table swaps cost
1.3us) and staged through aT's half-1 slots.  Window 3 is PE-bound: FFN1
half 1 with the deferred gelus interleaved, then FFN2 per half with the b2
bias folded in via a rank-1 ones matmul and the LN2 + output DMA fused per
half.

v4 refinements (each HW-validated correct at 8.52e-3, timing within run
noise of the 745-780us band): V projection restructured so quad pairs
share each stationary x-tile through a single 2-bank PSUM tile (keeps
the scheduler from splitting the chains) and a post-schedule pass strips
the then-redundant LDWEIGHTS (397 total, incl. FFN2's nh-pairs); the
LayerNorm rsqrt moved off the ACT engine (bit-trick + 2 Newton steps on
the DVE) so exp/gelu activation tables never swap against Sqrt; xT DMA
split per d-chunk so the first projections wait only on data they read.
Measured flat => the PE stream is already near its practical occupancy
for this schedule; LDWEIGHTS were being hidden by the background weight
buffer, and act-table swaps were off the critical path.

Precision (gate 2e-2; measured 8.5e-3 end-to-end on HW): QKV projections
and attn@V in fp8e4m3 DoubleRow (two 128-partition contraction tiles per
pass, 2x bf16 throughput - measured; the cost model's additional 2x for
DoubleRow does not materialize on hardware, the 256B/cycle input port is
the ceiling); q/k/exp(scores) stored fp8e4m3; scores and FFN matmuls in
bf16; PSUM always fp32; layernorm stats in fp32 over bf16 activations.
The softmax denominator falls out of attn@V via a ones-column on V (padded
to 68-wide blocks - dual-fp8 Ldweights requires 4-aligned stationary
columns); biases fold into matmuls or evictions; softmax uses a constant
shift (scores are O(+-6)).
"""

import contextlib
import os
import sys

import numpy as np

if "/opt/trn_rl_repo" not in sys.path and os.path.isdir("/opt/trn_rl_repo"):
    sys.path.insert(0, "/opt/trn_rl_repo")

B, S, D, H, DH, F = 4, 2048, 1024, 16, 64, 4096
N_CORES = 8
TOK = 1024  # query tokens per core
EPS = 1e-5
EXP_SHIFT = -3.0  # constant shift inside exp; cancels in softmax

_BUILD_CACHE = {}


def _strip_redundant_ldweights(nc, mybir):
    """Remove InstLdweights whose stationary operand is identical to the
    immediately preceding weight load (PE retains the stationary operand
    across matmuls).  Runs post-scheduling, pre-compile; only drops loads
    that carry no sync and whose key (AP+mode) matches the previous load
    with nothing but matmuls in between."""
    removed = 0
    for bb in nc.main_func.blocks:
        new = []
        last = None
        for inst in bb.instructions:
            if isinstance(inst, mybir.InstLdweights):
                pap = inst.ins[0]
                key = (pap.memref, pap.offset, str(pap.ap), str(pap.dtype),
                       str(inst.perf_mode), bool(inst.is_transpose),
                       str(inst.tile_position))
                si = inst.sync_info
                clean = si is None or (not si.on_wait and not si.on_update)
                if last == key and clean:
                    removed += 1
                    continue
                last = key
            elif isinstance(
                    inst, (mybir.InstMatmult, mybir.InstEventSemaphore,
                           mybir.InstNoOp)):
                # matmuls consume (not clobber) the loaded weights;
                # event-semaphores/nops only wait — all transparent to the
                # PE array's weight state
                pass
            elif getattr(inst, "engine", None) == mybir.EngineType.PE:
                last = None
            new.append(inst)
        bb.instructions = new
    return removed


def _build(n_iters=1, use_dp=True):
    import concourse.bacc as bacc
    import concourse.mybir as mybir
    import concourse.tile as tile
    from concourse.masks import make_identity
    from contextlib import ExitStack

    f32 = mybir.dt.float32
    bf16 = mybir.dt.bfloat16
    e4 = mybir.dt.float8e4
    AF = mybir.ActivationFunctionType
    DR = mybir.MatmulPerfMode.DoubleRow
    DP = mybir.MatmulPerfMode.DoublePixel if use_dp else None

    nc = bacc.Bacc("TRN2", target_bir_lowering=False, debug=False,
                   num_devices=N_CORES)

    xT = nc.dram_tensor("xT", [D, S], e4, kind="ExternalInput").ap()
    xh = nc.dram_tensor("xh", [TOK, D], bf16, kind="ExternalInput").ap()
    wq = nc.dram_tensor("wq", [D, D], e4, kind="ExternalInput").ap()
    wk = nc.dram_tensor("wk", [D, D], e4, kind="ExternalInput").ap()
    wv = nc.dram_tensor("wv", [D, 4, 272], e4, kind="ExternalInput").ap()
    bqk = nc.dram_tensor("bqk", [2, D], f32, kind="ExternalInput").ap()
    bv4 = nc.dram_tensor("bv4", [1, 4, 272], f32, kind="ExternalInput").ap()
    w1 = nc.dram_tensor("w1", [D, F], bf16, kind="ExternalInput").ap()
    b1d = nc.dram_tensor("b1", [F], f32, kind="ExternalInput").ap()
    w2 = nc.dram_tensor("w2", [F, D], bf16, kind="ExternalInput").ap()
    b2rh = nc.dram_tensor("b2rh", [1, D], bf16, kind="ExternalInput").ap()
    ln1g = nc.dram_tensor("ln1g", [D], bf16, kind="ExternalInput").ap()
    ln2g = nc.dram_tensor("ln2g", [D], bf16, kind="ExternalInput").ap()
    ln2b = nc.dram_tensor("ln2b", [D], bf16, kind="ExternalInput").ap()
    out = nc.dram_tensor("out", [TOK, D], f32, kind="ExternalOutput").ap()

    with tile.TileContext(nc) as tc, ExitStack() as top:
        const = top.enter_context(tc.tile_pool(name="const", bufs=1))
        ident_h = const.tile([128, 128], bf16)
        make_identity(nc, ident_h)
        eshift = const.tile([128, 1], f32)
        nc.vector.memset(eshift, EXP_SHIFT)
        eps_t = const.tile([128, 1], f32)
        nc.vector.memset(eps_t, EPS)
        bq_sb = const.tile([128, 8], f32)
        nc.sync.dma_start(out=bq_sb, in_=bqk[0].rearrange("(pr p) -> p pr", p=128))
        bk_sb = const.tile([128, 8], f32)
        nc.sync.dma_start(out=bk_sb, in_=bqk[1].rearrange("(pr p) -> p pr", p=128))
        b1_sb = const.tile([128, 32], f32)
        nc.sync.dma_start(out=b1_sb, in_=b1d.rearrange("(fc p) -> p fc", p=128))
        ones_r = const.tile([1, 128], bf16)
        nc.vector.memset(ones_r, 1.0)

        resid = top.enter_context(tc.tile_pool(name="resid", bufs=1))

        loop = tc.For_i(0, n_iters) if n_iters > 1 else contextlib.nullcontext()
        with loop:
            mha = resid.tile([128, 8, D], bf16, tag="mha")

            with ExitStack() as pha:
                att = pha.enter_context(tc.tile_pool(name="att", bufs=1))
                zpool = pha.enter_context(tc.tile_pool(name="zp", bufs=2))
                stream = pha.enter_context(tc.tile_pool(name="stream", bufs=3))
                stat = pha.enter_context(tc.tile_pool(name="stat", bufs=4))
                opool = pha.enter_context(tc.tile_pool(name="osb", bufs=2))

                kT_all = att.tile([128, 8, S], e4)
                qT_all = att.tile([128, 8, TOK], e4)
                vplus_all = att.tile([128, 4, 16, 272], e4)
                h1T = att.tile([128, 8, TOK], bf16)

                with tc.tile_pool(name="psA", bufs=1, space="PSUM") as psA:

                    def scores_part(head, sch, hook=None):
                        pair, h01 = head // 2, head % 2
                        pslice = slice(h01 * 64, h01 * 64 + 64)
                        expT = att.tile([128, 16, 512], e4, tag="expT",
                                        bufs=2)
                        for tp in range(8):
                            sT_ps = psA.tile([128, 1024], f32, tag="sT2",
                                             bufs=2)
                            for sub in range(2):
                                tt = 2 * tp + sub
                                nc.tensor.matmul(
                                    out=sT_ps[:, sub * 512:(sub + 1) * 512],
                                    lhsT=kT_all[pslice, pair,
                                                tt * 128:(tt + 1) * 128],
                                    rhs=qT_all[pslice, pair,
                                               sch * 512:(sch + 1) * 512],
                                    start=True, stop=True,
                                    perf_mode=DP)
                            nc.scalar.activation(
                                out=expT[:, 2 * tp:2 * tp + 2, :],
                                in_=sT_ps, func=AF.Exp,
                                bias=eshift[:, :], scale=0.125)
                            if hook is not None:
                                hook(tp)
                        return expT

                    def attnv_part(head, sch, expT):
                        pair, h01 = head // 2, head % 2
                        quad, j = pair // 2, 2 * (pair % 2) + h01
                        zT_ps = psA.tile([68, 512], f32, tag="zT", bufs=1)
                        for t2 in range(8):
                            nc.tensor.matmul(
                                out=zT_ps,
                                lhsT=vplus_all[:, quad, 2 * t2:2 * t2 + 2,
                                               68 * j:68 * j + 68],
                                rhs=expT[:, 2 * t2:2 * t2 + 2, :],
                                start=(t2 == 0), stop=(t2 == 7),
                                perf_mode=DR)
                        zT_sb = zpool.tile([68, 512], bf16, tag="zT_sb")
                        nc.vector.tensor_copy(out=zT_sb, in_=zT_ps)
                        for sb4 in range(4):
                            ztr = psA.tile([128, 68], bf16, tag="small",
                                           bufs=1)
                            nc.tensor.transpose(
                                out=ztr,
                                in_=zT_sb[:, sb4 * 128:(sb4 + 1) * 128],
                                identity=ident_h[0:68, 0:68])
                            rec = zpool.tile([128, 1], f32, tag="rec")
                            nc.vector.reciprocal(out=rec, in_=ztr[:, 64:65])
                            stg = sch * 4 + sb4
                            nc.vector.tensor_scalar_mul(
                                out=mha[:, stg, head * 64:head * 64 + 64],
                                in0=ztr[:, 0:64], scalar1=rec)

                    def attn_block(head, sch, hook=None):
                        attnv_part(head, sch, scores_part(head, sch, hook))

                    ph1 = ExitStack()
                    xpool = ph1.enter_context(tc.tile_pool(name="xT", bufs=1))
                    wp = ph1.enter_context(tc.tile_pool(name="wpair", bufs=2))

                    xT_sb = xpool.tile([128, 8, S], e4)
                    # split across both DMA queues AND per d-chunk so the
                    # first projection matmuls wait only on the chunks
                    # they actually read, not the full 2MB
                    for dt in range(4):
                        nc.sync.dma_start(
                            out=xT_sb[:, dt, :],
                            in_=xT.rearrange("(dt p) s -> p dt s",
                                             p=128)[:, dt, :])
                        nc.gpsimd.dma_start(
                            out=xT_sb[:, 4 + dt, :],
                            in_=xT.rearrange("(dt p) s -> p dt s",
                                             p=128)[:, 4 + dt, :])

                    # V biases + residual input + LN constants ride the
                    # gpsimd DMA queue so the sync queue serves xT / weights
                    bv_all4 = xpool.tile([128, 4, 272], f32, tag="bv4")
                    nc.gpsimd.dma_start(
                        out=bv_all4, in_=bv4[0, :, :].partition_broadcast(128))
                    xh_sb = resid.tile([128, 8, D], bf16, tag="xh")
                    g1_bc = resid.tile([128, D], bf16, tag="g1")
                    nc.gpsimd.dma_start(out=g1_bc,
                                        in_=ln1g.partition_broadcast(128))
                    g2_bc = resid.tile([128, D], bf16, tag="g2")
                    nc.gpsimd.dma_start(out=g2_bc,
                                        in_=ln2g.partition_broadcast(128))
                    b2_bc = resid.tile([128, D], bf16, tag="lb2")
                    nc.gpsimd.dma_start(out=b2_bc,
                                        in_=ln2b.partition_broadcast(128))
                    b2row = resid.tile([1, D], bf16, tag="b2row")
                    nc.gpsimd.dma_start(out=b2row, in_=b2rh[:, :])

                    # ---- window 1: projections + attention(half 0) ------
                    wvs = {}

                    def v_dma(quad):
                        wv_sb = wp.tile([128, 8, 272], e4, tag="wv", bufs=4)
                        nc.sync.dma_start(
                            out=wv_sb,
                            in_=wv[:, quad, :].rearrange(
                                "(dt p) c -> p dt c", p=128))
                        wvs[quad] = wv_sb

                    def v_piece_pair(qp, tt):
                        # quads 2qp/2qp+1 share each stationary x-tile.
                        # Both accumulation chains live in ONE 2-bank PSUM
                        # tile so they become ready together and the
                        # scheduler keeps their matmuls interleaved; the
                        # then-redundant weight reloads (213ns each, vs
                        # 57ns of streaming) get stripped post-schedule.
                        quads = (2 * qp, 2 * qp + 1)
                        vp = psA.tile([128, 2, 512], f32, tag="sT2",
                                      bufs=2, name=f"vpp_{qp}_{tt}")
                        for dp in range(4):
                            for i in range(2):
                                nc.tensor.matmul(
                                    out=vp[:, i, 0:272],
                                    lhsT=xT_sb[:, 2 * dp:2 * dp + 2,
                                               tt * 128:(tt + 1) * 128],
                                    rhs=wvs[quads[i]][:, 2 * dp:2 * dp + 2, :],
                                    start=(dp == 0), stop=(dp == 3),
                                    perf_mode=DR)
                        for i in range(2):
                            nc.vector.tensor_add(
                                out=vplus_all[:, quads[i], tt, :],
                                in0=vp[:, i, 0:272],
                                in1=bv_all4[:, quads[i], :])

                    def v_piece_all(tt):
                        v_piece_pair(0, tt)
                        v_piece_pair(1, tt)

                    def proj_pair(pair):
                        proj_pair_k(pair)
                        proj_pair_q(pair)

                    wqs = {}

                    def proj_pair_k(pair):
                        wk_sb = wp.tile([128, 8, 128], e4, tag="wk")
                        nc.sync.dma_start(
                            out=wk_sb,
                            in_=wk[:, pair * 128:(pair + 1) * 128]
                            .rearrange("(dt p) m -> p dt m", p=128))
                        wq_sb = wp.tile([128, 8, 128], e4, tag="wq")
                        nc.sync.dma_start(
                            out=wq_sb,
                            in_=wq[:, pair * 128:(pair + 1) * 128]
                            .rearrange("(dt p) m -> p dt m", p=128))
                        wqs[pair] = wq_sb
                        for ch in range(2):
                            kq_ps = psA.tile([128, 1024], f32,
                                             tag="sT2", bufs=2)
                            for nh in range(2):
                                for dp in range(4):
                                    nc.tensor.matmul(
                                        out=kq_ps[:, nh * 512:
                                                  (nh + 1) * 512],
                                        lhsT=wk_sb[:, 2 * dp:2 * dp + 2, :],
                                        rhs=xT_sb[:, 2 * dp:2 * dp + 2,
                                                  (2 * ch + nh) * 512:
                                                  (2 * ch + nh + 1) * 512],
                                        start=(dp == 0), stop=(dp == 3),
                                        perf_mode=DR)
                            nc.vector.tensor_scalar_add(
                                out=kT_all[:, pair,
                                           ch * 1024:(ch + 1) * 1024],
                                in0=kq_ps,
                                scalar1=bk_sb[:, pair:pair + 1])

                    def proj_pair_q(pair):
                        wq_sb = wqs.pop(pair)
                        kq_ps = psA.tile([128, 1024], f32, tag="sT2",
                                         bufs=2)
                        for nh in range(2):
                            for dp in range(4):
                                nc.tensor.matmul(
                                    out=kq_ps[:, nh * 512:(nh + 1) * 512],
                                    lhsT=wq_sb[:, 2 * dp:2 * dp + 2, :],
                                    rhs=xT_sb[:, 2 * dp:2 * dp + 2,
                                              nh * 512:(nh + 1) * 512],
                                    start=(dp == 0), stop=(dp == 3),
                                    perf_mode=DR)
                        nc.vector.tensor_scalar_add(
                            out=qT_all[:, pair, :], in0=kq_ps,
                            scalar1=bq_sb[:, pair:pair + 1])

                    # two-pair-ahead prefetch keeps the exp stream
                    # running; projection pieces are scattered between the
                    # score groups so the PE never sits >4us ahead of the
                    # act engine's exp drain.  All 16 V tt-pieces complete
                    # before the first attn@V (which consumes the full key
                    # range): tt 0-7 in the prologue, 8-15 inside pair 0.
                    proj_pair(0)
                    for quad in range(4):
                        v_dma(quad)
                    for tt in range(8):
                        v_piece_all(tt)
                    proj_pair(1)
                    for pair in range(8):
                        if pair == 4:
                            for st in range(8):
                                nc.gpsimd.dma_start(
                                    out=xh_sb[:, st, :],
                                    in_=xh[st * 128:(st + 1) * 128, :])
                        nxt = pair + 2
                        eT0 = scores_part(2 * pair, 0)
                        if pair == 0:
                            for tt in range(8, 12):
                                v_piece_all(tt)
                        if nxt < 8:
                            proj_pair_k(nxt)
                        eT1 = scores_part(2 * pair + 1, 0)
                        if pair == 0:
                            for tt in range(12, 16):
                                v_piece_all(tt)
                        if nxt < 8:
                            proj_pair_q(nxt)
                        attnv_part(2 * pair, 0, eT0)
                        attnv_part(2 * pair + 1, 0, eT1)

                    ph1.close()  # free xT + weight staging for aT

                    # ---- LN1 + h1T transposes + g1 fold, per half -------
                    def layer_norm_tile(st, g_bc, b_bc, add_in=None,
                                        add_ps=None, out_dma=False):
                        h = mha[:, st, :]
                        if add_in is not None:
                            nc.vector.tensor_add(out=h, in0=h, in1=add_in)
                        if add_ps is not None:
                            nc.vector.tensor_add(out=h, in0=h, in1=add_ps)
                        stats = stat.tile([128, 2, 6], f32, tag="stats")
                        for sg in range(2):
                            nc.vector.bn_stats(
                                out=stats[:, sg, :],
                                in_=h[:, sg * 512:(sg + 1) * 512])
                        mv = stat.tile([128, 2], f32, tag="mv")
                        nc.vector.bn_aggr(out=mv, in_=stats)
                        # rsqrt(var+eps) on the DVE (bit-trick seed + 2
                        # Newton steps, ~5e-6 rel err): keeps the Sqrt off
                        # the ACT engine, whose exp/gelu tables would
                        # otherwise thrash (no table set holds exp+sqrt;
                        # each swap costs ~1.3us and stalls the LN chain)
                        i32 = mybir.dt.int32
                        AL = mybir.AluOpType
                        vv = mv[:, 1:2]
                        ti = stat.tile([128, 1], i32, tag="ti")
                        qq = stat.tile([128, 1], f32, tag="qq")
                        y1 = stat.tile([128, 1], f32, tag="y1")
                        nc.vector.tensor_scalar(out=vv, in0=vv, scalar1=EPS,
                                                scalar2=None, op0=AL.add)
                        nc.vector.tensor_scalar(
                            out=ti, in0=vv.bitcast(i32), scalar1=1,
                            scalar2=None, op0=AL.logical_shift_right)
                        nc.vector.tensor_scalar(
                            out=ti, in0=ti, scalar1=-1, scalar2=0x5f3759df,
                            op0=AL.mult, op1=AL.add)
                        for nit in range(2):
                            yin = ti.bitcast(f32) if nit == 0 else y1
                            yout = y1 if nit == 0 else mv[:, 1:2]
                            nc.vector.tensor_tensor(out=qq, in0=vv, in1=yin,
                                                    op=AL.mult)
                            nc.vector.tensor_tensor(out=qq, in0=qq, in1=yin,
                                                    op=AL.mult)
                            nc.vector.tensor_scalar(
                                out=qq, in0=qq, scalar1=-0.5, scalar2=1.5,
                                op0=AL.mult, op1=AL.add)
                            nc.vector.tensor_tensor(out=yout, in0=yin,
                                                    in1=qq, op=AL.mult)
                        nc.vector.tensor_scalar(
                            out=h, in0=h, scalar1=mv[:, 0:1],
                            scalar2=mv[:, 1:2],
                            op0=mybir.AluOpType.subtract,
                            op1=mybir.AluOpType.mult)
                        if g_bc is not None:
                            nc.vector.tensor_mul(out=h, in0=h, in1=g_bc[:, :])
                            o_sb = opool.tile([128, D], f32, tag="osb")
                            nc.vector.tensor_add(out=o_sb, in0=h,
                                                 in1=b_bc[:, :])
                            if out_dma:
                                nc.sync.dma_start(
                                    out=out.rearrange(
                                        "(st p) d -> p st d", p=128)[:, st, :],
                                    in_=o_sb)

                    def b1_half(half):
                        for st in range(4 * half, 4 * half + 4):
                            layer_norm_tile(st, None, None,
                                            add_in=xh_sb[:, st, :])
                        for st in range(4 * half, 4 * half + 4):
                            for dt in range(8):
                                # alternate between the two idle PSUM banks
                                # (vp/ztr and zT are unused here) so the
                                # transpose->copy chains double-buffer
                                tr_ps = psA.tile([128, 128], bf16,
                                                 tag=("small" if dt % 2 == 0
                                                      else "zT"), bufs=1)
                                nc.tensor.transpose(
                                    out=tr_ps,
                                    in_=mha[:, st, dt * 128:(dt + 1) * 128],
                                    identity=ident_h[:, :])
                                nc.vector.tensor_copy(
                                    out=h1T[:, dt, st * 128:(st + 1) * 128],
                                    in_=tr_ps)
                        for st in range(4 * half, 4 * half + 4):
                            nc.vector.tensor_mul(out=mha[:, st, :],
                                                 in0=mha[:, st, :],
                                                 in1=g1_bc[:, :])

                    b1_half(0)

                    # ---- window 2: attention(half 1) + FFN1 half 0 ------
                    aTp = pha.enter_context(tc.tile_pool(name="aTp", bufs=1))
                    aT = aTp.tile([128, 32, TOK], bf16, tag="aT")
                    fstate = {}

                    def ffn1_chunk(fc, half, dts):
                        if dts[0] == 0:
                            w1t = stream.tile([128, 8, 128], bf16, tag="w1t",
                                              name=f"w1t_{half}_{fc}")
                            nc.sync.dma_start(
                                out=w1t,
                                in_=w1[:, fc * 128:(fc + 1) * 128].rearrange(
                                    "(dt p) f -> p dt f", p=128))
                            fstate["w1t"] = w1t
                            fstate["aps"] = psA.tile(
                                [128, 512], f32, tag="aps", bufs=2,
                                name=f"aps_{half}_{fc}")
                        a_ps = fstate["aps"]
                        for dt in dts:
                            nc.tensor.matmul(
                                out=a_ps,
                                lhsT=fstate["w1t"][:, dt, :],
                                rhs=h1T[:, dt,
                                        half * 512:(half + 1) * 512],
                                start=(dt == 0), stop=(dt == 7))
                        if dts[-1] == 7:
                            if half == 0:
                                # defer gelu out of the exp stream (the act
                                # table swap costs 1.3us); stage raw preact
                                # in the unused half-1 slot of aT
                                nc.vector.tensor_copy(
                                    out=aT[:, fc, 512:1024], in_=a_ps)
                            else:
                                nc.scalar.activation(
                                    out=aT[:, fc, 512:1024],
                                    in_=a_ps, func=AF.Gelu,
                                    bias=b1_sb[:, fc:fc + 1])

                    def w2_hook(head):
                        def hook(tp):
                            fc = 2 * head + tp // 4
                            dts = [2 * (tp % 4), 2 * (tp % 4) + 1]
                            ffn1_chunk(fc, 0, dts)
                        return hook

                    for head in range(16):
                        attn_block(head, 1, hook=w2_hook(head))

                    b1_half(1)

                    # ---- window 3a: FFN1 half 1, with the deferred half-0
                    # gelus interleaved per fc (same act table, no swaps)
                    for fc in range(32):
                        nc.scalar.activation(
                            out=aT[:, fc, 0:512], in_=aT[:, fc, 512:1024],
                            func=AF.Gelu, bias=b1_sb[:, fc:fc + 1])
                        for dts in ([0, 1], [2, 3], [4, 5], [6, 7]):
                            ffn1_chunk(fc, 1, dts)

                # ---- window 3b: FFN2 per half + fused LN2 + DMA ---------
                with tc.tile_pool(name="ps2", bufs=1, space="PSUM") as ps2:
                    for sh in range(2):
                        ffps = [ps2.tile([128, D], f32, tag="ff", bufs=4,
                                         name=f"ff_{sh}_{i}")
                                for i in range(4)]
                        for fc in range(32):
                            w2t = stream.tile([128, D], bf16, tag="w2t")
                            nc.sync.dma_start(
                                out=w2t, in_=w2[fc * 128:(fc + 1) * 128, :])
                            for st2 in range(4):
                                base = sh * 512 + st2 * 128
                                for nh in range(2):
                                    nc.tensor.matmul(
                                        out=ffps[st2][:, nh * 512:
                                                      (nh + 1) * 512],
                                        lhsT=aT[:, fc, base:base + 128],
                                        rhs=w2t[:, nh * 512:(nh + 1) * 512],
                                        start=(fc == 0), stop=False)
                        for st2 in range(4):
                            for nh in range(2):
                                nc.tensor.matmul(
                                    out=ffps[st2][:, nh * 512:(nh + 1) * 512],
                                    lhsT=ones_r[:, :],
                                    rhs=b2row[:, nh * 512:(nh + 1) * 512],
                                    start=False, stop=True)
                        for st2 in range(4):
                            layer_norm_tile(sh * 4 + st2, g2_bc, b2_bc,
                                            add_ps=ffps[st2][:, :],
                                            out_dma=True)

    _strip_redundant_ldweights(nc, mybir)
    nc.compile()
    return nc


def _pack_inputs(x, Wq, bq, Wk, bk, Wv, bv, ln1_g, ln1_b, W1, b1, W2, b2,
                 ln2_g, ln2_b):
    """Build the 8 per-core input maps (host-side, numpy)."""
    from concourse import mybir

    f = np.float32
    E4 = mybir.dt.np(mybir.dt.float8e4)
    BF = mybir.dt.np(mybir.dt.bfloat16)
    wq_all = np.ascontiguousarray(
        np.transpose(np.asarray(Wq, f), (1, 0, 2)).reshape(D, D)).astype(E4)
    wk_all = np.ascontiguousarray(
        np.transpose(np.asarray(Wk, f), (1, 0, 2)).reshape(D, D)).astype(E4)
    Wv_ = np.asarray(Wv, f)
    bv_ = np.asarray(bv, f)
    wv_all = np.zeros((D, 4, 272), f)
    bv_all = np.zeros((1, 4, 272), f)
    for quad in range(4):
        for j in range(4):
            h = quad * 4 + j
            wv_all[:, quad, 68 * j:68 * j + 64] = Wv_[h]
            bv_all[0, quad, 68 * j:68 * j + 64] = bv_[h]
            bv_all[0, quad, 68 * j + 64] = 1.0
    bqk = np.stack([np.asarray(bq, f).reshape(D), np.asarray(bk, f).reshape(D)])
    x = np.asarray(x, f)
    W1_ = np.asarray(W1, np.float64)
    g1_ = np.asarray(ln1_g, np.float64)
    bb1_ = np.asarray(ln1_b, np.float64)
    w1_folded = (g1_[:, None] * W1_).astype(BF)
    b1_folded = (np.asarray(b1, np.float64) + bb1_ @ W1_).astype(f)
    b2_folded = (np.asarray(b2, np.float64) + bb1_).astype(f)
    common = dict(
        wq=wq_all, wk=wk_all, wv=wv_all.astype(E4), bqk=bqk, bv4=bv_all,
        w1=w1_folded, b1=b1_folded, w2=np.asarray(W2, f).astype(BF),
        b2rh=b2_folded.reshape(1, D).astype(BF),
        ln1g=np.asarray(ln1_g, f).astype(BF),
        ln2g=np.asarray(ln2_g, f).astype(BF),
        ln2b=np.asarray(ln2_b, f).astype(BF))
    in_maps = []
    for c in range(N_CORES):
        b_, half = c // 2, c % 2
        m = dict(common)
        own = x[b_, half * TOK:(half + 1) * TOK]
        other = x[b_, (1 - half) * TOK:(2 - half) * TOK]
        m["xT"] = np.ascontiguousarray(
            np.concatenate([own, other], axis=0).T).astype(E4)
        m["xh"] = np.ascontiguousarray(own).astype(BF)
        in_maps.append(m)
    return in_maps


def kernel(**inputs):
    from concourse.bass_utils import run_bass_kernel_spmd

    if "nc" not in _BUILD_CACHE:
        _BUILD_CACHE["nc"] = _build()
    nc = _BUILD_CACHE["nc"]
    in_maps = _pack_inputs(**inputs)
    res = run_bass_kernel_spmd(nc, in_maps, core_ids=list(range(N_CORES)))
    out = np.zeros((B, S, D), np.float32)
    for c in range(N_CORES):
        b_, half = c // 2, c % 2
        out[b_, half * TOK:(half + 1) * TOK] = res.results[c]["out"]
    return out



# revision 28
# speedup vs baseline: 1.0745x; 1.0065x over previous
"""Trainium2 Bass kernel for nn_BackBone_77532749627801.

Transformer encoder block: per-head QKV projections -> MHA (softmax over
keys) -> AddNorm -> FFN (erf GELU) -> AddNorm.  B=4, S=2048, D=1024, H=16,
DH=64, F=4096.

Sharding: 8 cores = 4 batches x 2 sequence-halves; each core runs the block
for 1024 query tokens of one batch (K/V computed for the full 2048-token
sequence, so no collectives).  Host-side rotation puts each core's own
tokens in xT columns 0:1024.

Schedule: the softmax exp stream (33.6M elements) pins the activation
engine at ~250us, while the matmul stream pins the PE at ~530us, so the
kernel is organized to overlap them.  Window 1 computes QKV projections and
attention for query-half 0, with projections prefetched two head-pairs
ahead so the exp stream never starves (K/Q PSUM evictions run on the DVE to
keep the act engine on exp).  Window 2 runs attention for query-half 1 with
FFN1 chunks for half 0 interleaved between the score matmuls; the half-0
gelus are deferred out of the exp stream (activation-table swaps cost
1.3us) and staged through aT's half-1 slots.  Window 3 is PE-bound: FFN1
half 1 with the deferred gelus interleaved, then FFN2 per half with the b2
bias folded in via a rank-1 ones matmul and the LN2 + output DMA fused per
half.

v4 refinements (each HW-validated correct at 8.52e-3, timing within run
noise of the 745-780us band): V projection restructured so quad pairs
share each stationary x-tile through a single 2-bank PSUM tile (keeps
the scheduler from splitting the chains) and a post-schedule pass strips
the then-redundant LDWEIGHTS (397 total, incl. FFN2's nh-pairs); the
LayerNorm rsqrt moved off the ACT engine (bit-trick + 2 Newton steps on
the DVE) so exp/gelu activation tables never swap against Sqrt; xT DMA
split per d-chunk so the first projections wait only on data they read.
Measured flat => the PE stream is already near its practical occupancy
for this schedule; LDWEIGHTS were being hidden by the background weight
buffer, and act-table swaps were off the critical path.

Precision (gate 2e-2; measured 8.5e-3 end-to-end on HW): QKV projections
and attn@V in fp8e4m3 DoubleRow (two 128-partition contraction tiles per
pass, 2x bf16 throughput - measured; the cost model's additional 2x for
DoubleRow does not materialize on hardware, the 256B/cycle input port is
the ceiling); q/k/exp(scores) stored fp8e4m3; scores and FFN matmuls in
bf16; PSUM always fp32; layernorm stats in fp32 over bf16 activations.
The softmax denominator falls out of attn@V via a ones-column on V (padded
to 68-wide blocks - dual-fp8 Ldweights requires 4-aligned stationary
columns); biases fold into matmuls or evictions; softmax uses a constant
shift (scores are O(+-6)).
"""

import contextlib
import os
import sys

import numpy as np

if "/opt/trn_rl_repo" not in sys.path and os.path.isdir("/opt/trn_rl_repo"):
    sys.path.insert(0, "/opt/trn_rl_repo")

B, S, D, H, DH, F = 4, 2048, 1024, 16, 64, 4096
N_CORES = 8
TOK = 1024  # query tokens per core
EPS = 1e-5
EXP_SHIFT = -3.0  # constant shift inside exp; cancels in softmax

_BUILD_CACHE = {}


def _strip_redundant_ldweights(nc, mybir):
    """Remove InstLdweights whose stationary operand is identical to the
    immediately preceding weight load (PE retains the stationary operand
    across matmuls).  Runs post-scheduling, pre-compile; only drops loads
    that carry no sync and whose key (AP+mode) matches the previous load
    with nothing but matmuls in between."""
    removed = 0
    for bb in nc.main_func.blocks:
        new = []
        last = None
        for inst in bb.instructions:
            if isinstance(inst, mybir.InstLdweights):
                pap = inst.ins[0]
                key = (pap.memref, pap.offset, str(pap.ap), str(pap.dtype),
                       str(inst.perf_mode), bool(inst.is_transpose),
                       str(inst.tile_position))
                si = inst.sync_info
                clean = si is None or (not si.on_wait and not si.on_update)
                if last == key and clean:
                    removed += 1
                    continue
                last = key
            elif isinstance(
                    inst, (mybir.InstMatmult, mybir.InstEventSemaphore,
                           mybir.InstNoOp)):
                # matmuls consume (not clobber) the loaded weights;
                # event-semaphores/nops only wait — all transparent to the
                # PE array's weight state
                pass
            elif getattr(inst, "engine", None) == mybir.EngineType.PE:
                last = None
            new.append(inst)
        bb.instructions = new
    return removed


def _build(n_iters=1, use_dp=True):
    import concourse.bacc as bacc
    import concourse.mybir as mybir
    import concourse.tile as tile
    from concourse.masks import make_identity
    from contextlib import ExitStack

    f32 = mybir.dt.float32
    bf16 = mybir.dt.bfloat16
    e4 = mybir.dt.float8e4
    AF = mybir.ActivationFunctionType
    DR = mybir.MatmulPerfMode.DoubleRow
    DP = mybir.MatmulPerfMode.DoublePixel if use_dp else None

    nc = bacc.Bacc("TRN2", target_bir_lowering=False, debug=False,
                   num_devices=N_CORES)

    xT = nc.dram_tensor("xT", [D, S], e4, kind="ExternalInput").ap()
    xh = nc.dram_tensor("xh", [TOK, D], bf16, kind="ExternalInput").ap()
    wq = nc.dram_tensor("wq", [D, D], e4, kind="ExternalInput").ap()
    wk = nc.dram_tensor("wk", [D, D], e4, kind="ExternalInput").ap()
    wv = nc.dram_tensor("wv", [D, 4, 272], e4, kind="ExternalInput").ap()
    bqk = nc.dram_tensor("bqk", [2, D], f32, kind="ExternalInput").ap()
    bv4 = nc.dram_tensor("bv4", [1, 4, 272], f32, kind="ExternalInput").ap()
    w1 = nc.dram_tensor("w1", [D, F], bf16, kind="ExternalInput").ap()
    b1d = nc.dram_tensor("b1", [F], f32, kind="ExternalInput").ap()
    w2 = nc.dram_tensor("w2", [F, D], bf16, kind="ExternalInput").ap()
    b2rh = nc.dram_tensor("b2rh", [1, D], bf16, kind="ExternalInput").ap()
    ln1g = nc.dram_tensor("ln1g", [D], bf16, kind="ExternalInput").ap()
    ln2g = nc.dram_tensor("ln2g", [D], bf16, kind="ExternalInput").ap()
    ln2b = nc.dram_tensor("ln2b", [D], bf16, kind="ExternalInput").ap()
    out = nc.dram_tensor("out", [TOK, D], f32, kind="ExternalOutput").ap()

    with tile.TileContext(nc) as tc, ExitStack() as top:
        const = top.enter_context(tc.tile_pool(name="const", bufs=1))
        ident_h = const.tile([128, 128], bf16)
        make_identity(nc, ident_h)
        eshift = const.tile([128, 1], f32)
        nc.vector.memset(eshift, EXP_SHIFT)
        eps_t = const.tile([128, 1], f32)
        nc.vector.memset(eps_t, EPS)
        bq_sb = const.tile([128, 8], f32)
        nc.sync.dma_start(out=bq_sb, in_=bqk[0].rearrange("(pr p) -> p pr", p=128))
        bk_sb = const.tile([128, 8], f32)
        nc.sync.dma_start(out=bk_sb, in_=bqk[1].rearrange("(pr p) -> p pr", p=128))
        b1_sb = const.tile([128, 32], f32)
        nc.sync.dma_start(out=b1_sb, in_=b1d.rearrange("(fc p) -> p fc", p=128))
        ones_r = const.tile([1, 128], bf16)
        nc.vector.memset(ones_r, 1.0)

        resid = top.enter_context(tc.tile_pool(name="resid", bufs=1))

        loop = tc.For_i(0, n_iters) if n_iters > 1 else contextlib.nullcontext()
        with loop:
            mha = resid.tile([128, 8, D], bf16, tag="mha")

            with ExitStack() as pha:
                att = pha.enter_context(tc.tile_pool(name="att", bufs=1))
                zpool = pha.enter_context(tc.tile_pool(name="zp", bufs=2))
                stream = pha.enter_context(tc.tile_pool(name="stream", bufs=4))
                stat = pha.enter_context(tc.tile_pool(name="stat", bufs=4))
                opool = pha.enter_context(tc.tile_pool(name="osb", bufs=2))

                kT_all = att.tile([128, 8, S], e4)
                qT_all = att.tile([128, 8, TOK], e4)
                vplus_all = att.tile([128, 4, 16, 272], e4)
                h1T = att.tile([128, 8, TOK], bf16)

                with tc.tile_pool(name="psA", bufs=1, space="PSUM") as psA:

                    def scores_part(head, sch, hook=None):
                        pair, h01 = head // 2, head % 2
                        pslice = slice(h01 * 64, h01 * 64 + 64)
                        expT = att.tile([128, 16, 512], e4, tag="expT",
                                        bufs=2)
                        for tp in range(8):
                            sT_ps = psA.tile([128, 1024], f32, tag="sT2",
                                             bufs=2)
                            for sub in range(2):
                                tt = 2 * tp + sub
                                nc.tensor.matmul(
                                    out=sT_ps[:, sub * 512:(sub + 1) * 512],
                                    lhsT=kT_all[pslice, pair,
                                                tt * 128:(tt + 1) * 128],
                                    rhs=qT_all[pslice, pair,
                                               sch * 512:(sch + 1) * 512],
                                    start=True, stop=True,
                                    perf_mode=DP)
                            nc.scalar.activation(
                                out=expT[:, 2 * tp:2 * tp + 2, :],
                                in_=sT_ps, func=AF.Exp,
                                bias=eshift[:, :], scale=0.125)
                            if hook is not None:
                                hook(tp)
                        return expT

                    def attnv_part(head, sch, expT):
                        pair, h01 = head // 2, head % 2
                        quad, j = pair // 2, 2 * (pair % 2) + h01
                        zT_ps = psA.tile([68, 512], f32, tag="zT", bufs=1)
                        for t2 in range(8):
                            nc.tensor.matmul(
                                out=zT_ps,
                                lhsT=vplus_all[:, quad, 2 * t2:2 * t2 + 2,
                                               68 * j:68 * j + 68],
                                rhs=expT[:, 2 * t2:2 * t2 + 2, :],
                                start=(t2 == 0), stop=(t2 == 7),
                                perf_mode=DR)
                        zT_sb = zpool.tile([68, 512], bf16, tag="zT_sb")
                        nc.vector.tensor_copy(out=zT_sb, in_=zT_ps)
                        for sb4 in range(4):
                            ztr = psA.tile([128, 68], bf16, tag="small",
                                           bufs=1)
                            nc.tensor.transpose(
                                out=ztr,
                                in_=zT_sb[:, sb4 * 128:(sb4 + 1) * 128],
                                identity=ident_h[0:68, 0:68])
                            rec = zpool.tile([128, 1], f32, tag="rec")
                            nc.vector.reciprocal(out=rec, in_=ztr[:, 64:65])
                            stg = sch * 4 + sb4
                            nc.vector.tensor_scalar_mul(
                                out=mha[:, stg, head * 64:head * 64 + 64],
                                in0=ztr[:, 0:64], scalar1=rec)

                    def attn_block(head, sch, hook=None):
                        attnv_part(head, sch, scores_part(head, sch, hook))

                    ph1 = ExitStack()
                    xpool = ph1.enter_context(tc.tile_pool(name="xT", bufs=1))
                    wp = ph1.enter_context(tc.tile_pool(name="wpair", bufs=2))

                    xT_sb = xpool.tile([128, 8, S], e4)
                    # split across both DMA queues AND per d-chunk so the
                    # first projection matmuls wait only on the chunks
                    # they actually read, not the full 2MB
                    for dt in range(4):
                        nc.sync.dma_start(
                            out=xT_sb[:, dt, :],
                            in_=xT.rearrange("(dt p) s -> p dt s",
                                             p=128)[:, dt, :])
                        nc.gpsimd.dma_start(
                            out=xT_sb[:, 4 + dt, :],
                            in_=xT.rearrange("(dt p) s -> p dt s",
                                             p=128)[:, 4 + dt, :])

                    # V biases + residual input + LN constants ride the
                    # gpsimd DMA queue so the sync queue serves xT / weights
                    bv_all4 = xpool.tile([128, 4, 272], f32, tag="bv4")
                    nc.gpsimd.dma_start(
                        out=bv_all4, in_=bv4[0, :, :].partition_broadcast(128))
                    xh_sb = resid.tile([128, 8, D], bf16, tag="xh")
                    g1_bc = resid.tile([128, D], bf16, tag="g1")
                    nc.gpsimd.dma_start(out=g1_bc,
                                        in_=ln1g.partition_broadcast(128))
                    g2_bc = resid.tile([128, D], bf16, tag="g2")
                    nc.gpsimd.dma_start(out=g2_bc,
                                        in_=ln2g.partition_broadcast(128))
                    b2_bc = resid.tile([128, D], bf16, tag="lb2")
                    nc.gpsimd.dma_start(out=b2_bc,
                                        in_=ln2b.partition_broadcast(128))
                    b2row = resid.tile([1, D], bf16, tag="b2row")
                    nc.gpsimd.dma_start(out=b2row, in_=b2rh[:, :])

                    # ---- window 1: projections + attention(half 0) ------
                    wvs = {}

                    def v_dma(quad):
                        wv_sb = wp.tile([128, 8, 272], e4, tag="wv", bufs=4)
                        nc.sync.dma_start(
                            out=wv_sb,
                            in_=wv[:, quad, :].rearrange(
                                "(dt p) c -> p dt c", p=128))
                        wvs[quad] = wv_sb

                    def v_piece_pair(qp, tt):
                        # quads 2qp/2qp+1 share each stationary x-tile.
                        # Both accumulation chains live in ONE 2-bank PSUM
                        # tile so they become ready together and the
                        # scheduler keeps their matmuls interleaved; the
                        # then-redundant weight reloads (213ns each, vs
                        # 57ns of streaming) get stripped post-schedule.
                        quads = (2 * qp, 2 * qp + 1)
                        vp = psA.tile([128, 2, 512], f32, tag="sT2",
                                      bufs=2, name=f"vpp_{qp}_{tt}")
                        for dp in range(4):
                            for i in range(2):
                                nc.tensor.matmul(
                                    out=vp[:, i, 0:272],
                                    lhsT=xT_sb[:, 2 * dp:2 * dp + 2,
                                               tt * 128:(tt + 1) * 128],
                                    rhs=wvs[quads[i]][:, 2 * dp:2 * dp + 2, :],
                                    start=(dp == 0), stop=(dp == 3),
                                    perf_mode=DR)
                        for i in range(2):
                            nc.vector.tensor_add(
                                out=vplus_all[:, quads[i], tt, :],
                                in0=vp[:, i, 0:272],
                                in1=bv_all4[:, quads[i], :])

                    def v_piece_all(tt):
                        v_piece_pair(0, tt)
                        v_piece_pair(1, tt)

                    def proj_pair(pair):
                        proj_pair_k(pair)
                        proj_pair_q(pair)

                    wqs = {}

                    def proj_pair_k(pair):
                        wk_sb = wp.tile([128, 8, 128], e4, tag="wk")
                        nc.sync.dma_start(
                            out=wk_sb,
                            in_=wk[:, pair * 128:(pair + 1) * 128]
                            .rearrange("(dt p) m -> p dt m", p=128))
                        wq_sb = wp.tile([128, 8, 128], e4, tag="wq")
                        nc.sync.dma_start(
                            out=wq_sb,
                            in_=wq[:, pair * 128:(pair + 1) * 128]
                            .rearrange("(dt p) m -> p dt m", p=128))
                        wqs[pair] = wq_sb
                        for ch in range(2):
                            kq_ps = psA.tile([128, 1024], f32,
                                             tag="sT2", bufs=2)
                            for nh in range(2):
                                for dp in range(4):
                                    nc.tensor.matmul(
                                        out=kq_ps[:, nh * 512:
                                                  (nh + 1) * 512],
                                        lhsT=wk_sb[:, 2 * dp:2 * dp + 2, :],
                                        rhs=xT_sb[:, 2 * dp:2 * dp + 2,
                                                  (2 * ch + nh) * 512:
                                                  (2 * ch + nh + 1) * 512],
                                        start=(dp == 0), stop=(dp == 3),
                                        perf_mode=DR)
                            nc.vector.tensor_scalar_add(
                                out=kT_all[:, pair,
                                           ch * 1024:(ch + 1) * 1024],
                                in0=kq_ps,
                                scalar1=bk_sb[:, pair:pair + 1])

                    def proj_pair_q(pair):
                        wq_sb = wqs.pop(pair)
                        kq_ps = psA.tile([128, 1024], f32, tag="sT2",
                                         bufs=2)
                        for nh in range(2):
                            for dp in range(4):
                                nc.tensor.matmul(
                                    out=kq_ps[:, nh * 512:(nh + 1) * 512],
                                    lhsT=wq_sb[:, 2 * dp:2 * dp + 2, :],
                                    rhs=xT_sb[:, 2 * dp:2 * dp + 2,
                                              nh * 512:(nh + 1) * 512],
                                    start=(dp == 0), stop=(dp == 3),
                                    perf_mode=DR)
                        nc.vector.tensor_scalar_add(
                            out=qT_all[:, pair, :], in0=kq_ps,
                            scalar1=bq_sb[:, pair:pair + 1])

                    # two-pair-ahead prefetch keeps the exp stream
                    # running; projection pieces are scattered between the
                    # score groups so the PE never sits >4us ahead of the
                    # act engine's exp drain.  All 16 V tt-pieces complete
                    # before the first attn@V (which consumes the full key
                    # range): tt 0-7 in the prologue, 8-15 inside pair 0.
                    proj_pair(0)
                    for quad in range(4):
                        v_dma(quad)
                    for tt in range(8):
                        v_piece_all(tt)
                    proj_pair(1)
                    for pair in range(8):
                        if pair == 4:
                            for st in range(8):
                                nc.gpsimd.dma_start(
                                    out=xh_sb[:, st, :],
                                    in_=xh[st * 128:(st + 1) * 128, :])
                        nxt = pair + 2
                        eT0 = scores_part(2 * pair, 0)
                        if pair == 0:
                            for tt in range(8, 12):
                                v_piece_all(tt)
                        if nxt < 8:
                            proj_pair_k(nxt)
                        eT1 = scores_part(2 * pair + 1, 0)
                        if pair == 0:
                            for tt in range(12, 16):
                                v_piece_all(tt)
                        if nxt < 8:
                            proj_pair_q(nxt)
                        attnv_part(2 * pair, 0, eT0)
                        attnv_part(2 * pair + 1, 0, eT1)

                    ph1.close()  # free xT + weight staging for aT

                    # ---- LN1 + h1T transposes + g1 fold, per half -------
                    def layer_norm_tile(st, g_bc, b_bc, add_in=None,
                                        add_ps=None, out_dma=False):
                        h = mha[:, st, :]
                        if add_in is not None:
                            nc.vector.tensor_add(out=h, in0=h, in1=add_in)
                        if add_ps is not None:
                            nc.vector.tensor_add(out=h, in0=h, in1=add_ps)
                        stats = stat.tile([128, 2, 6], f32, tag="stats")
                        for sg in range(2):
                            nc.vector.bn_stats(
                                out=stats[:, sg, :],
                                in_=h[:, sg * 512:(sg + 1) * 512])
                        mv = stat.tile([128, 2], f32, tag="mv")
                        nc.vector.bn_aggr(out=mv, in_=stats)
                        # rsqrt(var+eps) on the DVE (bit-trick seed + 2
                        # Newton steps, ~5e-6 rel err): keeps the Sqrt off
                        # the ACT engine, whose exp/gelu tables would
                        # otherwise thrash (no table set holds exp+sqrt;
                        # each swap costs ~1.3us and stalls the LN chain)
                        i32 = mybir.dt.int32
                        AL = mybir.AluOpType
                        vv = mv[:, 1:2]
                        ti = stat.tile([128, 1], i32, tag="ti")
                        qq = stat.tile([128, 1], f32, tag="qq")
                        y1 = stat.tile([128, 1], f32, tag="y1")
                        nc.vector.tensor_scalar(out=vv, in0=vv, scalar1=EPS,
                                                scalar2=None, op0=AL.add)
                        nc.vector.tensor_scalar(
                            out=ti, in0=vv.bitcast(i32), scalar1=1,
                            scalar2=None, op0=AL.logical_shift_right)
                        nc.vector.tensor_scalar(
                            out=ti, in0=ti, scalar1=-1, scalar2=0x5f3759df,
                            op0=AL.mult, op1=AL.add)
                        for nit in range(2):
                            yin = ti.bitcast(f32) if nit == 0 else y1
                            yout = y1 if nit == 0 else mv[:, 1:2]
                            nc.vector.tensor_tensor(out=qq, in0=vv, in1=yin,
                                                    op=AL.mult)
                            nc.vector.tensor_tensor(out=qq, in0=qq, in1=yin,
                                                    op=AL.mult)
                            nc.vector.tensor_scalar(
                                out=qq, in0=qq, scalar1=-0.5, scalar2=1.5,
                                op0=AL.mult, op1=AL.add)
                            nc.vector.tensor_tensor(out=yout, in0=yin,
                                                    in1=qq, op=AL.mult)
                        nc.vector.tensor_scalar(
                            out=h, in0=h, scalar1=mv[:, 0:1],
                            scalar2=mv[:, 1:2],
                            op0=mybir.AluOpType.subtract,
                            op1=mybir.AluOpType.mult)
                        if g_bc is not None:
                            nc.vector.tensor_mul(out=h, in0=h, in1=g_bc[:, :])
                            o_sb = opool.tile([128, D], f32, tag="osb")
                            nc.vector.tensor_add(out=o_sb, in0=h,
                                                 in1=b_bc[:, :])
                            if out_dma:
                                nc.sync.dma_start(
                                    out=out.rearrange(
                                        "(st p) d -> p st d", p=128)[:, st, :],
                                    in_=o_sb)

                    def b1_half(half):
                        for st in range(4 * half, 4 * half + 4):
                            layer_norm_tile(st, None, None,
                                            add_in=xh_sb[:, st, :])
                        for st in range(4 * half, 4 * half + 4):
                            for dt in range(8):
                                # alternate between the two idle PSUM banks
                                # (vp/ztr and zT are unused here) so the
                                # transpose->copy chains double-buffer
                                tr_ps = psA.tile([128, 128], bf16,
                                                 tag=("small" if dt % 2 == 0
                                                      else "zT"), bufs=1)
                                nc.tensor.transpose(
                                    out=tr_ps,
                                    in_=mha[:, st, dt * 128:(dt + 1) * 128],
                                    identity=ident_h[:, :])
                                nc.vector.tensor_copy(
                                    out=h1T[:, dt, st * 128:(st + 1) * 128],
                                    in_=tr_ps)
                        for st in range(4 * half, 4 * half + 4):
                            nc.vector.tensor_mul(out=mha[:, st, :],
                                                 in0=mha[:, st, :],
                                                 in1=g1_bc[:, :])

                    b1_half(0)

                    # ---- window 2: attention(half 1) + FFN1 half 0 ------
                    aTp = pha.enter_context(tc.tile_pool(name="aTp", bufs=1))
                    aT = aTp.tile([128, 32, TOK], bf16, tag="aT")
                    fstate = {}

                    def ffn1_chunk(fc, half, dts):
                        if dts[0] == 0:
                            w1t = stream.tile([128, 8, 128], bf16, tag="w1t",
                                              name=f"w1t_{half}_{fc}")
                            # alternate fc chunks across both DMA queues:
                            # the gpsimd queue is otherwise idle here, and
                            # the FFN windows are weight-stream paced
                            q = nc.sync if fc % 2 == 0 else nc.gpsimd
                            q.dma_start(
                                out=w1t,
                                in_=w1[:, fc * 128:(fc + 1) * 128].rearrange(
                                    "(dt p) f -> p dt f", p=128))
                            fstate["w1t"] = w1t
                            fstate["aps"] = psA.tile(
                                [128, 512], f32, tag="aps", bufs=2,
                                name=f"aps_{half}_{fc}")
                        a_ps = fstate["aps"]
                        for dt in dts:
                            nc.tensor.matmul(
                                out=a_ps,
                                lhsT=fstate["w1t"][:, dt, :],
                                rhs=h1T[:, dt,
                                        half * 512:(half + 1) * 512],
                                start=(dt == 0), stop=(dt == 7))
                        if dts[-1] == 7:
                            if half == 0:
                                # defer gelu out of the exp stream (the act
                                # table swap costs 1.3us); stage raw preact
                                # in the unused half-1 slot of aT
                                nc.vector.tensor_copy(
                                    out=aT[:, fc, 512:1024], in_=a_ps)
                            else:
                                nc.scalar.activation(
                                    out=aT[:, fc, 512:1024],
                                    in_=a_ps, func=AF.Gelu,
                                    bias=b1_sb[:, fc:fc + 1])

                    def w2_hook(head):
                        def hook(tp):
                            fc = 2 * head + tp // 4
                            dts = [2 * (tp % 4), 2 * (tp % 4) + 1]
                            ffn1_chunk(fc, 0, dts)
                        return hook

                    for head in range(16):
                        attn_block(head, 1, hook=w2_hook(head))

                    b1_half(1)

                    # ---- window 3a: FFN1 half 1, with the deferred half-0
                    # gelus interleaved per fc (same act table, no swaps)
                    for fc in range(32):
                        nc.scalar.activation(
                            out=aT[:, fc, 0:512], in_=aT[:, fc, 512:1024],
                            func=AF.Gelu, bias=b1_sb[:, fc:fc + 1])
                        for dts in ([0, 1], [2, 3], [4, 5], [6, 7]):
                            ffn1_chunk(fc, 1, dts)

                # ---- window 3b: FFN2 per half + fused LN2 + DMA ---------
                with tc.tile_pool(name="ps2", bufs=1, space="PSUM") as ps2:
                    for sh in range(2):
                        ffps = [ps2.tile([128, D], f32, tag="ff", bufs=4,
                                         name=f"ff_{sh}_{i}")
                                for i in range(4)]
                        for fc in range(32):
                            w2t = stream.tile([128, D], bf16, tag="w2t")
                            q = nc.sync if fc % 2 == 0 else nc.gpsimd
                            q.dma_start(
                                out=w2t, in_=w2[fc * 128:(fc + 1) * 128, :])
                            for st2 in range(4):
                                base = sh * 512 + st2 * 128
                                for nh in range(2):
                                    nc.tensor.matmul(
                                        out=ffps[st2][:, nh * 512:
                                                      (nh + 1) * 512],
                                        lhsT=aT[:, fc, base:base + 128],
                                        rhs=w2t[:, nh * 512:(nh + 1) * 512],
                                        start=(fc == 0), stop=False)
                        for st2 in range(4):
                            for nh in range(2):
                                nc.tensor.matmul(
                                    out=ffps[st2][:, nh * 512:(nh + 1) * 512],
                                    lhsT=ones_r[:, :],
                                    rhs=b2row[:, nh * 512:(nh + 1) * 512],
                                    start=False, stop=True)
                        for st2 in range(4):
                            layer_norm_tile(sh * 4 + st2, g2_bc, b2_bc,
                                            add_ps=ffps[st2][:, :],
                                            out_dma=True)

    _strip_redundant_ldweights(nc, mybir)
    nc.compile()
    return nc


def _pack_inputs(x, Wq, bq, Wk, bk, Wv, bv, ln1_g, ln1_b, W1, b1, W2, b2,
                 ln2_g, ln2_b):
    """Build the 8 per-core input maps (host-side, numpy)."""
    from concourse import mybir

    f = np.float32
    E4 = mybir.dt.np(mybir.dt.float8e4)
    BF = mybir.dt.np(mybir.dt.bfloat16)
    wq_all = np.ascontiguousarray(
        np.transpose(np.asarray(Wq, f), (1, 0, 2)).reshape(D, D)).astype(E4)
    wk_all = np.ascontiguousarray(
        np.transpose(np.asarray(Wk, f), (1, 0, 2)).reshape(D, D)).astype(E4)
    Wv_ = np.asarray(Wv, f)
    bv_ = np.asarray(bv, f)
    wv_all = np.zeros((D, 4, 272), f)
    bv_all = np.zeros((1, 4, 272), f)
    for quad in range(4):
        for j in range(4):
            h = quad * 4 + j
            wv_all[:, quad, 68 * j:68 * j + 64] = Wv_[h]
            bv_all[0, quad, 68 * j:68 * j + 64] = bv_[h]
            bv_all[0, quad, 68 * j + 64] = 1.0
    bqk = np.stack([np.asarray(bq, f).reshape(D), np.asarray(bk, f).reshape(D)])
    x = np.asarray(x, f)
    W1_ = np.asarray(W1, np.float64)
    g1_ = np.asarray(ln1_g, np.float64)
    bb1_ = np.asarray(ln1_b, np.float64)
    w1_folded = (g1_[:, None] * W1_).astype(BF)
    b1_folded = (np.asarray(b1, np.float64) + bb1_ @ W1_).astype(f)
    b2_folded = (np.asarray(b2, np.float64) + bb1_).astype(f)
    common = dict(
        wq=wq_all, wk=wk_all, wv=wv_all.astype(E4), bqk=bqk, bv4=bv_all,
        w1=w1_folded, b1=b1_folded, w2=np.asarray(W2, f).astype(BF),
        b2rh=b2_folded.reshape(1, D).astype(BF),
        ln1g=np.asarray(ln1_g, f).astype(BF),
        ln2g=np.asarray(ln2_g, f).astype(BF),
        ln2b=np.asarray(ln2_b, f).astype(BF))
    in_maps = []
    for c in range(N_CORES):
        b_, half = c // 2, c % 2
        m = dict(common)
        own = x[b_, half * TOK:(half + 1) * TOK]
        other = x[b_, (1 - half) * TOK:(2 - half) * TOK]
        m["xT"] = np.ascontiguousarray(
            np.concatenate([own, other], axis=0).T).astype(E4)
        m["xh"] = np.ascontiguousarray(own).astype(BF)
        in_maps.append(m)
    return in_maps


def kernel(**inputs):
    from concourse.bass_utils import run_bass_kernel_spmd

    if "nc" not in _BUILD_CACHE:
        _BUILD_CACHE["nc"] = _build()
    nc = _BUILD_CACHE["nc"]
    in_maps = _pack_inputs(**inputs)
    res = run_bass_kernel_spmd(nc, in_maps, core_ids=list(range(N_CORES)))
    out = np.zeros((B, S, D), np.float32)
    for c in range(N_CORES):
        b_, half = c // 2, c % 2
        out[b_, half * TOK:(half + 1) * TOK] = res.results[c]["out"]
    return out

